# revision 15
# baseline (speedup 1.0000x reference)
"""3D Haar DWT (clean-mode subband stack) on 8 Trainium2 NeuronCores.

Problem (hardcoded): inputs (4, 128, 128, 128, 4) f32, A (128, 128) f32 Haar
analysis operator. Output (4, 64, 64, 64, 32) f32 = 8 subbands stacked on the
channel axis (LLL, LLH, LHL, LHH, HLL, HLH, HHL, HHH) x 4 channels.

Sharding: pure data parallel over (batch, d1-half): core k handles
b = k // 2, d1 range [64*(k%2), 64*(k%2)+64). The Haar transform is a 2-tap
non-overlapping filter (rows of A touch only columns 2i, 2i+1), so splitting
d1 on an even boundary requires no communication.

The whole device data path runs in bf16 (the rel-err budget is 2e-2; bf16
end-to-end lands ~5e-3), which halves both HBM streams vs f32 — HBM reads
and writes have largely independent bandwidth, so with both streams dense
the kernel is engine-cadence-bound, not DMA-bound. Host casts f32 -> bf16
while staging the per-core slabs and upcasts the bf16 result.

Per-core pipeline (slab pre-transposed on host, per chunk, to
[d2, t, d1, m, c] where d3 = 2m + t, i.e. even/odd d3 planes pre-split so
every DVE access is a contiguous bf16 run):
  1. DMA in chunks of 4-8 d1 slices (0.5-1 MiB, 4-8 KiB descriptors),
     partitions = d2. First/last chunks are half-size to compress the
     pipeline fill/drain.
  2. d3 butterfly on DVE: two whole-chunk contiguous tensor_tensor ops
     (bf16 2x packed mode).
  3. d2 transform as PE matmul (stationary bf16 +-0.5*A^T, FWL fast path),
     with the d1 butterfly folded into PSUM accumulation; 4 matmuls fill a
     2-bank [128, 1024] PSUM tile per d1 pair (small PSUM tiles keep the
     MM -> evac -> reuse loop shorter than the chunk cadence). ~10 dummy
     warm-up matmuls run during the first load so the PE HAM clock-gate is
     already at 8/8 when real matmuls start.
  4. One PSUM -> SBUF evacuation op per pair (FD=1024, pure copy + f32 ->
     bf16 cast; all scaling lives in the weights). ACT takes 3 pairs per
     full chunk, DVE the last — DVE casts are deferred past the next
     chunk's butterflies (the store rides along, since Tile dependencies
     follow emission order) to keep the DVE FIFO dense.
  5. One DMA out per chunk on SWDGE (so stores never head-of-line-block
     the load queue); host reassembles the subband-major layout.

Scale bookkeeping: reference applies A three times (factor s = 1/sqrt(2) per
nonzero). The d3/d1 butterflies apply +/-1 and the matmul applies
0.5*A = (0.5*s)*sign-pattern, so each path gets s^3 exactly as the
reference (the 0.5 supplies the two butterflies' missing s each).
"""

import sys

import numpy as np

if "/opt/trn_rl_repo" not in sys.path:
    sys.path.insert(0, "/opt/trn_rl_repo")

B, N, C = 4, 128, 4
N_CORES = 8
SLAB = 64          # d1 extent per core
MC = (N // 2) * C  # 256: contiguous (m, c) run per d3 parity plane
# (d1_start, d1_width) per chunk: a half-size first chunk compresses the
# pipeline fill; quarter-size final chunks compress the drain (the lag
# between the last load and the last store, when the DMA fabric idles).
CHUNKS = [(0, 4), (4, 8), (12, 8), (20, 8), (28, 8), (36, 8), (44, 8),
          (52, 8), (60, 2), (62, 2)]
N_WARMUP_MM = 10   # PE HAM warm-up matmuls issued during the first load

_BASS_CACHE = {}


def _haar_matrix():
    s = np.float32(1.0 / np.sqrt(2.0))
    A = np.zeros((N, N), dtype=np.float32)
    for i in range(N // 2):
        A[i, 2 * i] = s
        A[i, 2 * i + 1] = s
        A[64 + i, 2 * i] = -s
        A[64 + i, 2 * i + 1] = s
    return A


def _reference_numpy(inputs, A):
    # Fallback only: exact reference math on host (used if A is not Haar).
    x = np.einsum("ij,bpjqc->bpiqc", A, inputs)
    x = np.einsum("ij,bjpqc->bipqc", A, x)
    x = np.einsum("ij,bpqjc->bpqic", A, x)
    m = x.shape[1] // 2
    subs = [
        x[:, :m, :m, :m, :], x[:, :m, :m, m:, :],
        x[:, :m, m:, :m, :], x[:, :m, m:, m:, :],
        x[:, m:, :m, :m, :], x[:, m:, :m, m:, :],
        x[:, m:, m:, :m, :], x[:, m:, m:, m:, :],
    ]
    return np.concatenate(subs, axis=-1).astype(np.float32)


def _build_bass():
    import concourse.bacc as bacc
    import concourse.mybir as mybir
    import concourse.tile as tile

    f32 = mybir.dt.float32
    bf16 = mybir.dt.bfloat16

    # Bacc (not raw Bass): its compile() pipeline splits multi-sem waits into
    # EventSemaphore instructions — TRN2 instructions have one wait slot.
    nc = bacc.Bacc("TRN2", target_bir_lowering=False, debug=False)
    # x: per chunk, host-pre-transposed [d2 | t, d1, m*c] blocks (d3=2m+t)
    # concatenated along the free dim, so each load is one contiguous run
    # per partition and the d3 butterfly is two fully-contiguous ops.
    x = nc.dram_tensor("x", [N, SLAB * 2 * MC], bf16, kind="ExternalInput")
    atp = nc.dram_tensor("atp", [N, N], bf16, kind="ExternalInput")
    atn = nc.dram_tensor("atn", [N, N], bf16, kind="ExternalInput")
    # y: per chunk, [i2 | s1, pp_local, s3*o3*c] blocks at pair-offset
    # columns; i2 = s2*64 + o2. One contiguous store per chunk.
    y = nc.dram_tensor("y", [N, SLAB, 2 * MC], bf16, kind="ExternalOutput")

    with tile.TileContext(nc) as tc:
        with (
            tc.tile_pool(name="const", bufs=1) as cpool,
            tc.tile_pool(name="io", bufs=7) as tpool,
            tc.tile_pool(name="mid", bufs=5) as mpool,
            tc.tile_pool(name="psum", bufs=4, space="PSUM") as ppool,
        ):
            atp_sb = cpool.tile([N, N], bf16)
            atn_sb = cpool.tile([N, N], bf16)
            # Scratch operands for the HAM warm-up matmuls (values are
            # irrelevant; outputs are never read — but Tile requires reads
            # to see a write, so memset them on the otherwise-idle gpsimd).
            scr_w = cpool.tile([N, N], bf16)
            scr_r = cpool.tile([N, 512], bf16)
            nc.gpsimd.memset(scr_w[:], 0)
            nc.gpsimd.memset(scr_r[:], 0)

            pswarm = ppool.tile([N, 2 * 2 * MC], f32, tag="ps")
            for _ in range(N_WARMUP_MM):
                nc.tensor.matmul(pswarm[:, :512], lhsT=scr_w[:], rhs=scr_r[:],
                                 start=True, stop=True)

            # Deferred work from full chunks: the DVE evacuation (and the
            # store that must follow it — Tile dependencies follow emission
            # order) is emitted only after the NEXT chunk's butterflies, so
            # the DVE FIFO never stalls a butterfly behind a PSUM-dependent
            # cast.
            pending = []

            def flush_pending():
                for dst, src, store_out, store_in in pending:
                    nc.vector.tensor_copy(out=dst, in_=src)
                    nc.gpsimd.dma_start(out=store_out, in_=store_in)
                pending.clear()

            for ci, (st, w) in enumerate(CHUNKS):
                npair = w // 2
                # 1. load chunk: [d2 | t, d1, m*c] — one DMA, contiguous
                # 2*w*MC-element run per partition.
                off = st * 2 * MC
                T = tpool.tile([N, 2, w, MC], bf16, tag="T")
                if ci == 0:
                    # tiny const loads first so they never interrupt the
                    # bulk load stream
                    nc.sync.dma_start(out=atp_sb[:], in_=atp[:, :])
                    nc.sync.dma_start(out=atn_sb[:], in_=atn[:, :])
                nc.sync.dma_start(out=T[:], in_=x[:, off:off + 2 * w * MC])

                # 2. d3 butterfly: W[:, 0] = even+odd (low),
                # W[:, 1] = odd-even (high). Fully contiguous bf16 runs ->
                # DVE 2x packed mode.
                W = mpool.tile([N, 2, w, MC], bf16, tag="W")
                nc.vector.tensor_add(out=W[:, 0], in0=T[:, 0], in1=T[:, 1])
                nc.vector.tensor_sub(out=W[:, 1], in0=T[:, 1], in1=T[:, 0])

                flush_pending()

                # staging: (s1, o1_local, s3*o3*c)
                Yst = mpool.tile([N, 2, npair, 2 * MC], bf16, tag="Yst")

                for pp in range(npair):
                    # rhs views for this d1 pair; free order (k=s3, m, c)
                    # matches the subband split layout.
                    r0 = W[:, :, 2 * pp + 0]
                    r1 = W[:, :, 2 * pp + 1]
                    # One 2-bank PSUM tile per pair: [lo | hi].
                    ps = ppool.tile([N, 2 * 2 * MC], f32, tag="ps")
                    # 3. d2 transform + d1 butterfly in PSUM. atp runs
                    # first (3 matmuls), then atn — 2 weight loads per pair.
                    mm = nc.tensor.matmul
                    mm(ps[:, 0 * 512:1 * 512], lhsT=atp_sb[:], rhs=r0,
                       start=True, stop=False)
                    mm(ps[:, 0 * 512:1 * 512], lhsT=atp_sb[:], rhs=r1,
                       start=False, stop=True)
                    mm(ps[:, 1 * 512:2 * 512], lhsT=atp_sb[:], rhs=r1,
                       start=True, stop=False)
                    mm(ps[:, 1 * 512:2 * 512], lhsT=atn_sb[:], rhs=r0,
                       start=False, stop=True)
                    # 4. one evacuation op per pair: psum layout (s1,
                    # s3*m*c) matches the Yst slice.
                    dst = Yst[:, :, pp]
                    src = ps[:].rearrange("p (a f) -> p a f", a=2)
                    if w == 8 and pp == npair - 1:
                        pending.append(
                            (dst, src, y[:, st:st + w], Yst[:]))
                    else:
                        nc.scalar.copy(dst, src)

                # 5. one store per chunk on SWDGE (gpsimd) so stores never
                # head-of-line-block the load queue on the SP sequencer.
                # (full chunks: store rides with the deferred DVE cast)
                if w != 8:
                    nc.gpsimd.dma_start(out=y[:, st:st + w], in_=Yst[:])

            flush_pending()
    nc.compile()
    return nc


def make_in_maps(x, A):
    """Stage per-core inputs: per chunk, transpose the slab block to
    [d2, t, d1, m, c] (d3 = 2m + t) and concatenate chunk blocks along the
    free dim; cast to bf16. Weights are +-0.5*A^T in bf16."""
    import ml_dtypes

    atp = np.ascontiguousarray((0.5 * A.T).astype(ml_dtypes.bfloat16))
    atn = np.ascontiguousarray((-0.5 * A.T).astype(ml_dtypes.bfloat16))
    in_maps = []
    for k in range(N_CORES):
        b, h = divmod(k, 2)
        slab = x[b, h * SLAB:(h + 1) * SLAB]          # [d1, d2, d3, c]
        parts = []
        for st, w in CHUNKS:
            blk = (
                slab[st:st + w]                        # [w, d2, d3, c]
                .transpose(1, 0, 2, 3)                 # [d2, w, d3, c]
                .reshape(N, w, N // 2, 2, C)           # [d2, w, m, t, c]
                .transpose(0, 3, 1, 2, 4)              # [d2, t, w, m, c]
                .reshape(N, 2 * w * MC)
            )
            parts.append(blk)
        pre = np.concatenate(parts, axis=1)
        in_maps.append(
            {
                "x": np.ascontiguousarray(pre.astype(ml_dtypes.bfloat16)),
                "atp": atp,
                "atn": atn,
            }
        )
    return in_maps


def assemble_out(results):
    """Reassemble per-core y buffers (per chunk: [i2 | s1, pp_local,
    s3, o3, c] blocks, bf16) into the full (B, 64, 64, 64, 32) f32
    output."""
    out = np.empty((B, 64, 64, 64, 8 * C), np.float32)
    for k in range(N_CORES):
        b, h = divmod(k, 2)
        ybuf = results[k]["y"].astype(np.float32)      # [128, 64, 512]
        for st, w in CHUNKS:
            npair = w // 2
            pc = st // 2
            blk = ybuf[:, st:st + w].reshape(
                2, 64, 2, npair, 2, 64, C
            )  # (s2, o2, s1, ppl, s3, o3, c)
            out[b, 32 * h + pc:32 * h + pc + npair] = (
                blk.transpose(3, 1, 5, 2, 0, 4, 6)  # (ppl, o2, o3, s1, s2, s3, c)
                .reshape(npair, 64, 64, 8 * C)
            )
    return out


def kernel(**inputs):
    x = np.ascontiguousarray(np.asarray(inputs["inputs"], dtype=np.float32))
    A = np.asarray(inputs["A"], dtype=np.float32)
    assert x.shape == (B, N, N, N, C), x.shape

    if not np.allclose(A, _haar_matrix(), atol=1e-5):
        # Kernel hardcodes the 2-tap Haar structure; fall back for generic A.
        return _reference_numpy(x, A)

    from concourse.bass_utils import run_bass_kernel_spmd

    if "nc" not in _BASS_CACHE:
        _BASS_CACHE["nc"] = _build_bass()
    nc = _BASS_CACHE["nc"]

    in_maps = make_in_maps(x, A)
    res = run_bass_kernel_spmd(nc, in_maps, core_ids=list(range(N_CORES)))
    return assemble_out(res.results)


# revision 22
# speedup vs baseline: 1.0862x; 1.0862x over previous
"""3D Haar DWT (clean-mode subband stack) on 8 Trainium2 NeuronCores.

Problem (hardcoded): inputs (4, 128, 128, 128, 4) f32, A (128, 128) f32 Haar
analysis operator. Output (4, 64, 64, 64, 32) f32 = 8 subbands stacked on the
channel axis (LLL, LLH, LHL, LHH, HLL, HLH, HHL, HHH) x 4 channels.

Sharding: pure data parallel over (batch, d1-half): core k handles
b = k // 2, d1 range [64*(k%2), 64*(k%2)+64). The Haar transform is a 2-tap
non-overlapping filter (rows of A touch only columns 2i, 2i+1), so splitting
d1 on an even boundary requires no communication.

The whole device data path runs in bf16 (the rel-err budget is 2e-2; bf16
end-to-end lands ~5e-3), which halves both HBM streams vs f32 — HBM reads
and writes have largely independent bandwidth, so with both streams dense
the kernel is engine-cadence-bound, not DMA-bound. Host casts f32 -> bf16
while staging the per-core slabs and upcasts the bf16 result.

Per-core pipeline (slab pre-transposed on host, per chunk, to
[d2, t, d1, m, c] where d3 = 2m + t, i.e. even/odd d3 planes pre-split so
every DVE access is a contiguous bf16 run):
  1. DMA in chunks of 4-8 d1 slices (0.5-1 MiB, 4-8 KiB descriptors),
     partitions = d2. First/last chunks are half-size to compress the
     pipeline fill/drain.
  2. d3 butterfly on DVE: two whole-chunk contiguous tensor_tensor ops
     (bf16 2x packed mode).
  3. d2 transform as PE matmul (stationary bf16 +-0.5*A^T, FWL fast path),
     with the d1 butterfly folded into PSUM accumulation; 4 matmuls fill a
     2-bank [128, 1024] PSUM tile per d1 pair (small PSUM tiles keep the
     MM -> evac -> reuse loop shorter than the chunk cadence). ~10 dummy
     warm-up matmuls run during the first load so the PE HAM clock-gate is
     already at 8/8 when real matmuls start.
  4. One PSUM -> SBUF evacuation op per pair (FD=1024, pure copy + f32 ->
     bf16 cast; all scaling lives in the weights). ACT takes 3 pairs per
     full chunk, DVE the last — DVE casts are deferred past the next
     chunk's butterflies (the store rides along, since Tile dependencies
     follow emission order) to keep the DVE FIFO dense.
  5. One DMA out per chunk on SWDGE (so stores never head-of-line-block
     the load queue); host reassembles the subband-major layout.

Scale bookkeeping: reference applies A three times (factor s = 1/sqrt(2) per
nonzero). The d3/d1 butterflies apply +/-1 and the matmul applies
0.5*A = (0.5*s)*sign-pattern, so each path gets s^3 exactly as the
reference (the 0.5 supplies the two butterflies' missing s each).
"""

import sys

import numpy as np

if "/opt/trn_rl_repo" not in sys.path:
    sys.path.insert(0, "/opt/trn_rl_repo")

B, N, C = 4, 128, 4
N_CORES = 8
SLAB = 64          # d1 extent per core
MC = (N // 2) * C  # 256: contiguous (m, c) run per d3 parity plane
# (d1_start, d1_width) per chunk: half-size edge chunks compress the
# pipeline fill and drain.
CHUNKS = [(0, 4), (4, 8), (12, 8), (20, 8), (28, 8), (36, 8), (44, 8),
          (52, 8), (60, 4)]
# Each store is FIFO-gated behind a 1-element gpsimd read of the load
# STORE_LAG chunks ahead: stores only enter the DMA fabric as loads retire,
# so the shared SBUF-AXI fabric (~426 B/ns total) serves the
# compute-critical load stream first and drains the store backlog in the
# post-compute window.
STORE_LAG = 3
N_WARMUP_MM = 10   # PE HAM warm-up matmuls issued during the first load

_BASS_CACHE = {}


def _haar_matrix():
    s = np.float32(1.0 / np.sqrt(2.0))
    A = np.zeros((N, N), dtype=np.float32)
    for i in range(N // 2):
        A[i, 2 * i] = s
        A[i, 2 * i + 1] = s
        A[64 + i, 2 * i] = -s
        A[64 + i, 2 * i + 1] = s
    return A


def _reference_numpy(inputs, A):
    # Fallback only: exact reference math on host (used if A is not Haar).
    x = np.einsum("ij,bpjqc->bpiqc", A, inputs)
    x = np.einsum("ij,bjpqc->bipqc", A, x)
    x = np.einsum("ij,bpqjc->bpqic", A, x)
    m = x.shape[1] // 2
    subs = [
        x[:, :m, :m, :m, :], x[:, :m, :m, m:, :],
        x[:, :m, m:, :m, :], x[:, :m, m:, m:, :],
        x[:, m:, :m, :m, :], x[:, m:, :m, m:, :],
        x[:, m:, m:, :m, :], x[:, m:, m:, m:, :],
    ]
    return np.concatenate(subs, axis=-1).astype(np.float32)


def _build_bass():
    import concourse.bacc as bacc
    import concourse.mybir as mybir
    import concourse.tile as tile

    f32 = mybir.dt.float32
    bf16 = mybir.dt.bfloat16

    # Bacc (not raw Bass): its compile() pipeline splits multi-sem waits into
    # EventSemaphore instructions — TRN2 instructions have one wait slot.
    nc = bacc.Bacc("TRN2", target_bir_lowering=False, debug=False)
    # x: per chunk, host-pre-transposed [d2 | t, d1, m*c] blocks (d3=2m+t)
    # concatenated along the free dim, so each load is one contiguous run
    # per partition and the d3 butterfly is two fully-contiguous ops.
    x = nc.dram_tensor("x", [N, SLAB * 2 * MC], bf16, kind="ExternalInput")
    atp = nc.dram_tensor("atp", [N, N], bf16, kind="ExternalInput")
    atn = nc.dram_tensor("atn", [N, N], bf16, kind="ExternalInput")
    # y: per chunk, [i2 | s1, pp_local, s3*o3*c] blocks at pair-offset
    # columns; i2 = s2*64 + o2. One contiguous store per chunk.
    y = nc.dram_tensor("y", [N, SLAB, 2 * MC], bf16, kind="ExternalOutput")

    with tile.TileContext(nc) as tc:
        with (
            tc.tile_pool(name="const", bufs=1) as cpool,
            tc.tile_pool(name="io", bufs=7) as tpool,
            tc.tile_pool(name="mid", bufs=4) as mpool,
            tc.tile_pool(name="psum", bufs=4, space="PSUM") as ppool,
        ):
            atp_sb = cpool.tile([N, N], bf16)
            atn_sb = cpool.tile([N, N], bf16)
            # Scratch operands for the HAM warm-up matmuls (values are
            # irrelevant; outputs are never read — but Tile requires reads
            # to see a write, so memset them on the otherwise-idle gpsimd).
            scr_w = cpool.tile([N, N], bf16)
            scr_r = cpool.tile([N, 512], bf16)
            scr_g = cpool.tile([N, 1], bf16)
            nc.gpsimd.memset(scr_w[:], 0)
            nc.gpsimd.memset(scr_r[:], 0)

            pswarm = ppool.tile([N, 2 * 2 * MC], f32, tag="ps")
            for _ in range(N_WARMUP_MM):
                nc.tensor.matmul(pswarm[:, :512], lhsT=scr_w[:], rhs=scr_r[:],
                                 start=True, stop=True)

            # Deferred DVE evacuations from full chunks, emitted only after
            # the NEXT chunk's butterflies so the DVE FIFO never stalls a
            # butterfly behind a PSUM-dependent cast. (Tile dependencies
            # follow emission order, so each chunk's store is emitted after
            # its deferred cast.)
            pending_cast = []

            def flush_casts():
                for dst, src in pending_cast:
                    nc.vector.tensor_copy(out=dst, in_=src)
                pending_cast.clear()

            # Stores awaiting their load-gate: (store_out, store_in).
            store_queue = []
            T_tiles = []

            def emit_store(gate_T):
                store_out, store_in = store_queue.pop(0)
                if gate_T is not None:
                    nc.gpsimd.tensor_copy(out=scr_g[:],
                                          in_=gate_T[:, 0, 0, 0:1])
                nc.gpsimd.dma_start(out=store_out, in_=store_in)

            for ci, (st, w) in enumerate(CHUNKS):
                npair = w // 2
                # 1. load chunk: [d2 | t, d1, m*c] — one DMA, contiguous
                # 2*w*MC-element run per partition.
                off = st * 2 * MC
                T = tpool.tile([N, 2, w, MC], bf16, tag="T")
                T_tiles.append(T)
                if ci == 0:
                    # tiny const loads first so they never interrupt the
                    # bulk load stream
                    nc.sync.dma_start(out=atp_sb[:], in_=atp[:, :])
                    nc.sync.dma_start(out=atn_sb[:], in_=atn[:, :])
                nc.sync.dma_start(out=T[:], in_=x[:, off:off + 2 * w * MC])
                # release a lagged store, gated on THIS chunk's load
                if ci >= STORE_LAG and store_queue:
                    emit_store(T)

                # 2. d3 butterfly: W[:, 0] = even+odd (low),
                # W[:, 1] = odd-even (high). Fully contiguous bf16 runs ->
                # DVE 2x packed mode.
                W = mpool.tile([N, 2, w, MC], bf16, tag="W")
                nc.vector.tensor_add(out=W[:, 0], in0=T[:, 0], in1=T[:, 1])
                nc.vector.tensor_sub(out=W[:, 1], in0=T[:, 1], in1=T[:, 0])

                flush_casts()

                # staging: (s1, o1_local, s3*o3*c)
                Yst = mpool.tile([N, 2, npair, 2 * MC], bf16, tag="Yst")

                for pp in range(npair):
                    # rhs views for this d1 pair; free order (k=s3, m, c)
                    # matches the subband split layout.
                    r0 = W[:, :, 2 * pp + 0]
                    r1 = W[:, :, 2 * pp + 1]
                    # One 2-bank PSUM tile per pair: [lo | hi].
                    ps = ppool.tile([N, 2 * 2 * MC], f32, tag="ps")
                    # 3. d2 transform + d1 butterfly in PSUM. atp runs
                    # first (3 matmuls), then atn — 2 weight loads per pair.
                    mm = nc.tensor.matmul
                    mm(ps[:, 0 * 512:1 * 512], lhsT=atp_sb[:], rhs=r0,
                       start=True, stop=False)
                    mm(ps[:, 0 * 512:1 * 512], lhsT=atp_sb[:], rhs=r1,
                       start=False, stop=True)
                    mm(ps[:, 1 * 512:2 * 512], lhsT=atp_sb[:], rhs=r1,
                       start=True, stop=False)
                    mm(ps[:, 1 * 512:2 * 512], lhsT=atn_sb[:], rhs=r0,
                       start=False, stop=True)
                    # 4. one evacuation op per pair: psum layout (s1,
                    # s3*m*c) matches the Yst slice.
                    dst = Yst[:, :, pp]
                    src = ps[:].rearrange("p (a f) -> p a f", a=2)
                    if w == 8 and pp == npair - 1:
                        pending_cast.append((dst, src))
                    else:
                        nc.scalar.copy(dst, src)

                # 5. one store per chunk on SWDGE (gpsimd) so stores never
                # head-of-line-block the load queue on the SP sequencer;
                # queued here, released by a later chunk's load gate.
                store_queue.append((y[:, st:st + w], Yst[:]))

            flush_casts()
            while store_queue:
                emit_store(None)
    nc.compile()
    return nc


def make_in_maps(x, A):
    """Stage per-core inputs: per chunk, transpose the slab block to
    [d2, t, d1, m, c] (d3 = 2m + t) and concatenate chunk blocks along the
    free dim; cast to bf16. Weights are +-0.5*A^T in bf16."""
    import ml_dtypes

    atp = np.ascontiguousarray((0.5 * A.T).astype(ml_dtypes.bfloat16))
    atn = np.ascontiguousarray((-0.5 * A.T).astype(ml_dtypes.bfloat16))
    in_maps = []
    for k in range(N_CORES):
        b, h = divmod(k, 2)
        slab = x[b, h * SLAB:(h + 1) * SLAB]          # [d1, d2, d3, c]
        parts = []
        for st, w in CHUNKS:
            blk = (
                slab[st:st + w]                        # [w, d2, d3, c]
                .transpose(1, 0, 2, 3)                 # [d2, w, d3, c]
                .reshape(N, w, N // 2, 2, C)           # [d2, w, m, t, c]
                .transpose(0, 3, 1, 2, 4)              # [d2, t, w, m, c]
                .reshape(N, 2 * w * MC)
            )
            parts.append(blk)
        pre = np.concatenate(parts, axis=1)
        in_maps.append(
            {
                "x": np.ascontiguousarray(pre.astype(ml_dtypes.bfloat16)),
                "atp": atp,
                "atn": atn,
            }
        )
    return in_maps


def assemble_out(results):
    """Reassemble per-core y buffers (per chunk: [i2 | s1, pp_local,
    s3, o3, c] blocks, bf16) into the full (B, 64, 64, 64, 32) f32
    output."""
    out = np.empty((B, 64, 64, 64, 8 * C), np.float32)
    for k in range(N_CORES):
        b, h = divmod(k, 2)
        ybuf = results[k]["y"].astype(np.float32)      # [128, 64, 512]
        for st, w in CHUNKS:
            npair = w // 2
            pc = st // 2
            blk = ybuf[:, st:st + w].reshape(
                2, 64, 2, npair, 2, 64, C
            )  # (s2, o2, s1, ppl, s3, o3, c)
            out[b, 32 * h + pc:32 * h + pc + npair] = (
                blk.transpose(3, 1, 5, 2, 0, 4, 6)  # (ppl, o2, o3, s1, s2, s3, c)
                .reshape(npair, 64, 64, 8 * C)
            )
    return out


def kernel(**inputs):
    x = np.ascontiguousarray(np.asarray(inputs["inputs"], dtype=np.float32))
    A = np.asarray(inputs["A"], dtype=np.float32)
    assert x.shape == (B, N, N, N, C), x.shape

    if not np.allclose(A, _haar_matrix(), atol=1e-5):
        # Kernel hardcodes the 2-tap Haar structure; fall back for generic A.
        return _reference_numpy(x, A)

    from concourse.bass_utils import run_bass_kernel_spmd

    if "nc" not in _BASS_CACHE:
        _BASS_CACHE["nc"] = _build_bass()
    nc = _BASS_CACHE["nc"]

    in_maps = make_in_maps(x, A)
    res = run_bass_kernel_spmd(nc, in_maps, core_ids=list(range(N_CORES)))
    return assemble_out(res.results)
